# revision 11
# baseline (speedup 1.0000x reference)
"""Trainium2 Bass kernel for a pre-norm transformer decoder layer.

Module: B=2, T=2048, EMB=768, 12 heads x head_dim 768 (QKV proj 768->9216),
causal attention, out_proj 9216->768, MLP 768->3072->768 (exact gelu),
pre-LN on both sublayers, residual adds.

Sharding over 8 NeuronCores: core c = (b, g) with b = c // 4 (batch),
g = c % 4 (head-group of 3 heads).  Each core:
  LN1 -> qT/kT (transposed proj) + v for its 3 heads -> causal attention
  -> partial out_proj (its heads' slice of Wo) -> on-device ReduceScatter
  (groups [0-3], [4-7]) so core (b,g) holds token rows [512g, 512g+512) of
  the attention sublayer output -> residual + LN2 + MLP on that slice
  -> returns [512, 768].  Host concatenates.

Numerics: attention pipeline in bf16 operands (fp32 psum accum), MLP
matmuls in float32r, layernorm / softmax / residuals in fp32.
"""

import numpy as np
import ml_dtypes

EMB = 768
HEADS = 12
HD = 768
QKV = 9216
FF = 3072
T = 2048
B = 2
N_CORES = 8
HPG = 3              # heads per group
GW = HPG * HD        # 2304, group slice of QKV dim
EPS = 1e-5
SCALE = 1.0 / 96.0   # 1/sqrt(9216)
NEG = -30000.0       # additive causal mask value (exp -> 0)
TSLICE = T // 4      # 512 tokens per core after reduce-scatter

ET = EMB // 128      # 6   e-tiles
CT = GW // 128       # 18  c-tiles per group
TT = T // 128        # 16  t-tiles
TC = T // 512        # 4   t-chunks
FT = FF // 128       # 24  f-tiles
ZT = TSLICE // 128   # 4   z-tiles

_RUNNER = None


def _mask_ext_np():
    """[128, 896] extended causal mask.  Pattern k (k=0..3) for an S^T tile
    [s=128k+i, t=j (512 wide)] is mask_ext[:, 384-128k : 896-128k]:
    allowed (0.0) iff 128k + i <= j, else NEG."""
    m = np.full((128, 896), NEG, np.float32)
    for i in range(128):
        m[i, 384 + i:] = 0.0
    return m


def _build_nc():
    import concourse.bacc as bacc
    import concourse.tile as tile
    from concourse import mybir

    f32 = mybir.dt.float32
    bf16 = mybir.dt.bfloat16
    f32r = mybir.dt.float32r
    AF = mybir.ActivationFunctionType
    OP = mybir.AluOpType

    nc = bacc.Bacc("TRN2", target_bir_lowering=False, debug=False,
                   num_devices=N_CORES)

    # ---- I/O ----
    xT_d = nc.dram_tensor("xT", [EMB, T], f32, kind="ExternalInput")
    xres_d = nc.dram_tensor("xres", [TSLICE, EMB], f32, kind="ExternalInput")
    wq_d = nc.dram_tensor("wq", [EMB, GW], bf16, kind="ExternalInput")
    wk_d = nc.dram_tensor("wk", [EMB, GW], bf16, kind="ExternalInput")
    wv_d = nc.dram_tensor("wv", [EMB, GW], bf16, kind="ExternalInput")
    wo_d = nc.dram_tensor("wo", [GW, EMB], bf16, kind="ExternalInput")
    w1_d = nc.dram_tensor("w1", [EMB, FF], f32r, kind="ExternalInput")
    w2_d = nc.dram_tensor("w2", [FF, EMB], f32r, kind="ExternalInput")
    # vectors pre-reshaped on host: [128, n] with col j = v[128j:128(j+1)]
    b1_d = nc.dram_tensor("b1v", [128, FT], f32, kind="ExternalInput")
    g1_d = nc.dram_tensor("g1v", [128, ET], f32, kind="ExternalInput")
    be1_d = nc.dram_tensor("be1v", [128, ET], f32, kind="ExternalInput")
    g2_d = nc.dram_tensor("g2v", [128, ET], f32, kind="ExternalInput")
    be2_d = nc.dram_tensor("be2v", [128, ET], f32, kind="ExternalInput")
    bo_d = nc.dram_tensor("bov", [1, EMB], f32, kind="ExternalInput")
    b2_d = nc.dram_tensor("b2v", [1, EMB], f32, kind="ExternalInput")
    out_d = nc.dram_tensor("out", [TSLICE, EMB], f32, kind="ExternalOutput")

    # ---- consts / internal DRAM ----
    cmask_c = nc.inline_tensor(_mask_ext_np(), name="cmask")
    ident_c = nc.inline_tensor(np.eye(128, dtype=np.float32), name="ident")
    ones_f_c = nc.inline_tensor(np.ones((128, 1), np.float32), name="ones_f")
    ones_b_c = nc.inline_tensor(
        np.ones((128, 1), dtype=ml_dtypes.bfloat16), name="ones_b")

    vsc_d = nc.dram_tensor("vsc", [HPG, T, HD], bf16)        # v scratch
    ysc_d = nc.dram_tensor("ysc", [CT, 128, T], bf16)        # yT scratch
    rsin_d = nc.dram_tensor("rs_in", [T, EMB], f32)          # out_proj partial
    rsout_d = nc.dram_tensor("rs_out", [TSLICE, EMB], f32)   # reduced slice

    with tile.TileContext(nc) as tc:
        _build_body(nc, tc, mybir, locals())
    nc.compile()
    return nc


def _build_body(nc, tc, mybir, d):
    import contextlib

    f32 = mybir.dt.float32
    bf16 = mybir.dt.bfloat16
    f32r = mybir.dt.float32r
    AF = mybir.ActivationFunctionType
    OP = mybir.AluOpType

    xT_d = d["xT_d"]; xres_d = d["xres_d"]
    wq_d = d["wq_d"]; wk_d = d["wk_d"]; wv_d = d["wv_d"]; wo_d = d["wo_d"]
    w1_d = d["w1_d"]; w2_d = d["w2_d"]
    b1_d = d["b1_d"]; g1_d = d["g1_d"]; be1_d = d["be1_d"]
    g2_d = d["g2_d"]; be2_d = d["be2_d"]; bo_d = d["bo_d"]; b2_d = d["b2_d"]
    out_d = d["out_d"]
    cmask_c = d["cmask_c"]; ident_c = d["ident_c"]
    ones_f_c = d["ones_f_c"]; ones_b_c = d["ones_b_c"]
    vsc_d = d["vsc_d"]; ysc_d = d["ysc_d"]
    rsin_d = d["rsin_d"]; rsout_d = d["rsout_d"]

    es = contextlib.ExitStack()
    with es:
        const = es.enter_context(tc.tile_pool(name="const", bufs=1))
        rows = es.enter_context(tc.tile_pool(name="rows", bufs=1))

        # ---------- constants ----------
        cmask = const.tile([128, 896], f32, tag="cmask")
        nc.sync.dma_start(cmask[:], cmask_c[:, :])
        ident = const.tile([128, 128], f32, tag="ident")
        nc.sync.dma_start(ident[:], ident_c[:, :])
        ones_b = const.tile([128, 1], bf16, tag="ones_b")
        nc.sync.dma_start(ones_b[:], ones_b_c[:, :])

        def load_cols(dram, n, tag):
            # host pre-reshaped [128, n], col j = v[128j:128j+128]
            t_ = const.tile([128, n], f32, tag=tag, name=tag)
            nc.sync.dma_start(t_[:, :], dram[:, :])
            return t_

        g1c = load_cols(g1_d, ET, "g1c")
        be1c = load_cols(be1_d, ET, "be1c")
        g2c = load_cols(g2_d, ET, "g2c")
        be2c = load_cols(be2_d, ET, "be2c")
        b1c = load_cols(b1_d, FT, "b1c")

        # ---------- P0: LN1 over full batch sequence ----------
        ht_pool = es.enter_context(tc.tile_pool(name="ht", bufs=1))
        ht = [ht_pool.tile([128, T], bf16, tag=f"ht{j}", name=f"ht{j}")
          for j in range(ET)]

        with tc.tile_pool(name="p0", bufs=1) as p0, \
                tc.tile_pool(name="p0ps", bufs=1, space="PSUM") as p0ps:
            xt = []
            for j in range(ET):
                t_ = p0.tile([128, T], f32, tag=f"xt{j}")
                nc.sync.dma_start(t_[:], xT_d[128 * j:128 * (j + 1), :])
                xt.append(t_)

            mu_ps = [p0ps.tile([1, 512], f32, tag=f"mups{c}", name=f"mups{c}")
                     for c in range(TC)]
            ss_ps = [p0ps.tile([1, 512], f32, tag=f"ssps{c}", name=f"ssps{c}")
                     for c in range(TC)]
            for j in range(ET):
                xb = p0.tile([128, T], bf16, tag="xb")
                nc.vector.tensor_copy(xb[:], xt[j][:])
                sq = p0.tile([128, T], bf16, tag="sq")
                nc.scalar.square(sq[:], xt[j][:])
                for c in range(TC):
                    cs = slice(512 * c, 512 * (c + 1))
                    nc.tensor.matmul(
                        mu_ps[c][:], ones_b[:], xb[:, cs],
                        start=(j == 0), stop=(j == ET - 1))
                    nc.tensor.matmul(
                        ss_ps[c][:], ones_b[:], sq[:, cs],
                        start=(j == 0), stop=(j == ET - 1))

            mu_b = p0.tile([128, T], f32, tag="mu_b")
            rstd_b = p0.tile([128, T], f32, tag="rstd_b")
            for c in range(TC):
                cs = slice(512 * c, 512 * (c + 1))
                mrow = rows.tile([1, 512], f32, tag="r_mu")
                nc.scalar.mul(mrow[:], mu_ps[c][:], 1.0 / EMB)
                vrow = rows.tile([1, 512], f32, tag="r_var")
                nc.scalar.mul(vrow[:], ss_ps[c][:], 1.0 / EMB)
                nc.gpsimd.partition_broadcast(mu_b[:, cs], mrow[:])
                nc.vector.tensor_mul(mrow[:], mrow[:], mrow[:])
                nc.vector.tensor_sub(vrow[:], vrow[:], mrow[:])
                nc.vector.tensor_scalar_add(vrow[:], vrow[:], EPS)
                nc.scalar.activation(vrow[:], vrow[:], AF.Sqrt)
                nc.vector.reciprocal(vrow[:], vrow[:])
                nc.gpsimd.partition_broadcast(rstd_b[:, cs], vrow[:])

            for j in range(ET):
                t1 = p0.tile([128, T], f32, tag="ln1t")
                nc.vector.tensor_sub(t1[:], xt[j][:], mu_b[:])
                nc.vector.tensor_mul(t1[:], t1[:], rstd_b[:])
                nc.vector.tensor_scalar(
                    out=ht[j][:], in0=t1[:],
                    scalar1=g1c[:, j:j + 1], scalar2=be1c[:, j:j + 1],
                    op0=OP.mult, op1=OP.add)

        # ---------- attention region ----------
        with tc.tile_pool(name="att", bufs=1) as att, \
                tc.tile_pool(name="attw", bufs=2) as attw, \
                tc.tile_pool(name="attps", bufs=1, space="PSUM") as attps:

            def acc512(): return attps.tile([128, 512], f32,
                                             tag="acc512", bufs=2,
                                             name="acc512")
            def acc256(): return attps.tile([128, 256], f32,
                                             tag="acc512", bufs=2,
                                             name="acc256")

            qT = [att.tile([128, T], bf16, tag=f"qT{i}", name=f"qT{i}")
                  for i in range(ET)]
            kT = [att.tile([128, T], bf16, tag=f"kT{i}", name=f"kT{i}")
                  for i in range(ET)]

            for h in range(HPG):
                hs = slice(HD * h, HD * (h + 1))

                # --- projections ---
                for (dst, wsrc) in ((qT, wq_d), (kT, wk_d)):
                    wt = []
                    for j in range(ET):
                        w_ = attw.tile([128, HD], bf16, tag=f"wph{j}")
                        nc.sync.dma_start(
                            w_[:], wsrc[128 * j:128 * (j + 1), hs])
                        wt.append(w_)
                    for ci in range(ET):
                        csl = slice(128 * ci, 128 * (ci + 1))
                        for tp in range(2):
                            pa = acc512()
                            pb = acc512()
                            for j in range(ET):
                                nc.tensor.matmul(
                                    pa[:], wt[j][:, csl],
                                    ht[j][:, 1024 * tp:1024 * tp + 512],
                                    start=(j == 0), stop=(j == ET - 1))
                                nc.tensor.matmul(
                                    pb[:], wt[j][:, csl],
                                    ht[j][:, 1024 * tp + 512:1024 * (tp + 1)],
                                    start=(j == 0), stop=(j == ET - 1))
                            nc.vector.tensor_copy(
                                dst[ci][:, 1024 * tp:1024 * tp + 512], pa[:])
                            nc.vector.tensor_copy(
                                dst[ci][:, 1024 * tp + 512:1024 * (tp + 1)],
                                pb[:])

                wvh = []
                for j in range(ET):
                    w_ = attw.tile([128, HD], bf16, tag=f"wph{j}")
                    nc.sync.dma_start(w_[:], wv_d[128 * j:128 * (j + 1), hs])
                    wvh.append(w_)
                for i in range(TT):
                    ts_ = slice(128 * i, 128 * (i + 1))
                    p0_ = acc512()
                    p1_ = acc256()
                    for j in range(ET):
                        nc.tensor.matmul(p0_[:], ht[j][:, ts_],
                                         wvh[j][:, 0:512],
                                         start=(j == 0), stop=(j == ET - 1))
                        nc.tensor.matmul(p1_[:], ht[j][:, ts_],
                                         wvh[j][:, 512:768],
                                         start=(j == 0), stop=(j == ET - 1))
                    vt_ = attw.tile([128, HD], bf16, tag="vstage")
                    nc.vector.tensor_copy(vt_[:, 0:512], p0_[:])
                    nc.vector.tensor_copy(vt_[:, 512:768], p1_[:])
                    nc.sync.dma_start(vsc_d[h, ts_, :], vt_[:])

                # --- attention ---
                for tcn in range(TC):
                    ns = 4 * (tcn + 1)
                    tsl = slice(512 * tcn, 512 * (tcn + 1))
                    pts = []
                    for si in range(ns):
                        ssl = slice(128 * si, 128 * (si + 1))
                        ps_ = acc512()
                        for di in range(ET):
                            nc.tensor.matmul(
                                ps_[:], kT[di][:, ssl], qT[di][:, tsl],
                                start=(di == 0), stop=(di == ET - 1))
                        k_ = si - 4 * tcn
                        if k_ >= 0:
                            mo = 384 - 128 * k_
                            nc.vector.tensor_add(
                                ps_[:], ps_[:], cmask[:, mo:mo + 512])
                        pt_ = att.tile([128, 512], bf16, tag=f"pt{si}")
                        nc.scalar.activation(pt_[:], ps_[:], AF.Exp)
                        pts.append(pt_)
                    pden = attps.tile([1, 512], f32, tag="accY", bufs=6)
                    for si in range(ns):
                        nc.tensor.matmul(pden[:], ones_b[:], pts[si][:],
                                         start=(si == 0), stop=(si == ns - 1))
                    den = rows.tile([1, 512], f32, tag="r_den")
                    nc.vector.reciprocal(den[:], pden[:])
                    rb = attw.tile([128, 512], f32, tag="rb")
                    nc.gpsimd.partition_broadcast(rb[:], den[:])

                    pys = [attps.tile([128, 512], f32, tag="accY", bufs=6,
                                      name=f"py{_d}") for _d in range(ET)]
                    for si in range(ns):
                        ssl = slice(128 * si, 128 * (si + 1))
                        vt_ = attw.tile([128, HD], bf16, tag="vt", bufs=3)
                        nc.sync.dma_start(vt_[:], vsc_d[h, ssl, :])
                        for di in range(ET):
                            nc.tensor.matmul(
                                pys[di][:], vt_[:, 128 * di:128 * (di + 1)],
                                pts[si][:],
                                start=(si == 0), stop=(si == ns - 1))
                    for di in range(ET):
                        yst = attw.tile([128, 512], bf16, tag="ystg")
                        nc.vector.tensor_tensor(
                            out=yst[:], in0=pys[di][:], in1=rb[:], op=OP.mult)
                        nc.sync.dma_start(
                            ysc_d[ET * h + di, :, tsl], yst[:])

        # ---------- out_proj partial -> rs_in ----------
        with tc.tile_pool(name="p3", bufs=1) as p3, \
                tc.tile_pool(name="p3s", bufs=2) as p3s, \
                tc.tile_pool(name="p3ps", bufs=2, space="PSUM") as p3ps:
            wot = []
            for ci in range(CT):
                w_ = p3.tile([128, EMB], bf16, tag=f"wot{ci}")
                nc.sync.dma_start(w_[:], wo_d[128 * ci:128 * (ci + 1), :])
                wot.append(w_)
            for i in range(TT):
                ts_ = slice(128 * i, 128 * (i + 1))
                p0_ = p3ps.tile([128, 512], f32, tag="po0")
                p1_ = p3ps.tile([128, 256], f32, tag="po1")
                for ci in range(CT):
                    y_ = p3s.tile([128, 128], bf16, tag=f"yld{ci % 3}")
                    nc.sync.dma_start(y_[:], ysc_d[ci, :, ts_])
                    nc.tensor.matmul(p0_[:], y_[:], wot[ci][:, 0:512],
                                     start=(ci == 0), stop=(ci == CT - 1))
                    nc.tensor.matmul(p1_[:], y_[:], wot[ci][:, 512:768],
                                     start=(ci == 0), stop=(ci == CT - 1))
                ost = p3s.tile([128, EMB], f32, tag="ost")
                nc.vector.tensor_copy(ost[:, 0:512], p0_[:])
                nc.vector.tensor_copy(ost[:, 512:768], p1_[:])
                nc.sync.dma_start(rsin_d[ts_, :], ost[:])

        # ---------- reduce-scatter ----------
        nc.gpsimd.collective_compute(
            "ReduceScatter", OP.add,
            replica_groups=[[0, 1, 2, 3], [4, 5, 6, 7]],
            ins=[rsin_d.ap().opt()], outs=[rsout_d.ap().opt()])

        # ---------- residual + LN2 + MLP ----------
        with tc.tile_pool(name="p5", bufs=1) as p5, \
                tc.tile_pool(name="p5s", bufs=2) as p5s:
            bo_row = rows.tile([1, EMB], f32, tag="bo_row")
            nc.sync.dma_start(bo_row[:, :], bo_d[:, :])
            b2_row = rows.tile([1, EMB], f32, tag="b2_row")
            nc.sync.dma_start(b2_row[:, :], b2_d[:, :])
            bo_b = p5.tile([128, EMB], f32, tag="bo_b")
            nc.gpsimd.partition_broadcast(bo_b[:], bo_row[:])
            b2_b = p5.tile([128, EMB], f32, tag="b2_b")
            nc.gpsimd.partition_broadcast(b2_b[:], b2_row[:])

            z = []
            for i in range(ZT):
                ts_ = slice(128 * i, 128 * (i + 1))
                rl = p5s.tile([128, EMB], f32, tag="rsld")
                nc.sync.dma_start(rl[:], rsout_d[ts_, :])
                xr = p5s.tile([128, EMB], f32, tag="xrld")
                nc.sync.dma_start(xr[:], xres_d[ts_, :])
                z_ = p5.tile([128, EMB], f32, tag=f"z{i}")
                nc.vector.tensor_add(z_[:], rl[:], xr[:])
                nc.vector.tensor_add(z_[:], z_[:], bo_b[:])
                z.append(z_)

            h2 = [p5.tile([128, TSLICE], f32r, tag=f"h2{j}", name=f"h2{j}")
                  for j in range(ET)]
            with tc.tile_pool(name="ln2", bufs=1) as ln2, \
                    tc.tile_pool(name="ln2ps", bufs=1, space="PSUM") as ln2ps:
                zTt = [ln2.tile([128, TSLICE], f32, tag=f"zT{j}", name=f"zT{j}")
                       for j in range(ET)]
                for i in range(ZT):
                    for j in range(ET):
                        pt_ = ln2ps.tile([128, 128], f32, tag="ptr", bufs=2)
                        nc.tensor.transpose(
                            pt_[:], z[i][:, 128 * j:128 * (j + 1)], ident[:])
                        nc.vector.tensor_copy(
                            zTt[j][:, 128 * i:128 * (i + 1)], pt_[:])
                pmu2 = ln2ps.tile([1, 512], f32, tag="pmu2")
                pss2 = ln2ps.tile([1, 512], f32, tag="pss2")
                for j in range(ET):
                    zb = ln2.tile([128, TSLICE], bf16, tag="zb", bufs=2)
                    nc.vector.tensor_copy(zb[:], zTt[j][:])
                    sq = ln2.tile([128, TSLICE], bf16, tag="sq2", bufs=2)
                    nc.scalar.square(sq[:], zTt[j][:])
                    nc.tensor.matmul(pmu2[:], ones_b[:], zb[:],
                                     start=(j == 0), stop=(j == ET - 1))
                    nc.tensor.matmul(pss2[:], ones_b[:], sq[:],
                                     start=(j == 0), stop=(j == ET - 1))
                mrow = rows.tile([1, 512], f32, tag="r_mu")
                nc.scalar.mul(mrow[:], pmu2[:], 1.0 / EMB)
                vrow = rows.tile([1, 512], f32, tag="r_var")
                nc.scalar.mul(vrow[:], pss2[:], 1.0 / EMB)
                mu2_b = ln2.tile([128, TSLICE], f32, tag="mu2_b")
                nc.gpsimd.partition_broadcast(mu2_b[:], mrow[:])
                nc.vector.tensor_mul(mrow[:], mrow[:], mrow[:])
                nc.vector.tensor_sub(vrow[:], vrow[:], mrow[:])
                nc.vector.tensor_scalar_add(vrow[:], vrow[:], EPS)
                nc.scalar.activation(vrow[:], vrow[:], AF.Sqrt)
                nc.vector.reciprocal(vrow[:], vrow[:])
                rstd2_b = ln2.tile([128, TSLICE], f32, tag="rstd2_b")
                nc.gpsimd.partition_broadcast(rstd2_b[:], vrow[:])
                for j in range(ET):
                    t1 = ln2.tile([128, TSLICE], f32, tag="ln2t", bufs=2)
                    nc.vector.tensor_sub(t1[:], zTt[j][:], mu2_b[:])
                    nc.vector.tensor_mul(t1[:], t1[:], rstd2_b[:])
                    nc.vector.tensor_scalar(
                        out=h2[j][:], in0=t1[:],
                        scalar1=g2c[:, j:j + 1], scalar2=be2c[:, j:j + 1],
                        op0=OP.mult, op1=OP.add)

            # a1 = h2 @ W1 + b1 -> gelu -> g1a
            g1a = [p5.tile([128, TSLICE], f32r, tag=f"g1a{ft}",
                           name=f"g1a{ft}") for ft in range(FT)]
            with tc.tile_pool(name="a1w", bufs=2) as a1w, \
                    tc.tile_pool(name="a1ps", bufs=2, space="PSUM") as a1ps:
                for fc in range(FF // 512):
                    w1t = []
                    for j in range(ET):
                        w_ = a1w.tile([128, 512], f32r, tag=f"w1t{j}")
                        nc.sync.dma_start(
                            w_[:], w1_d[128 * j:128 * (j + 1),
                                        512 * fc:512 * (fc + 1)])
                        w1t.append(w_)
                    for sub in range(4):
                        ft = 4 * fc + sub
                        pa = a1ps.tile([128, 512], f32, tag="pa1")
                        for j in range(ET):
                            nc.tensor.matmul(
                                pa[:],
                                w1t[j][:, 128 * sub:128 * (sub + 1)],
                                h2[j][:],
                                start=(j == 0), stop=(j == ET - 1))
                        ga = a1w.tile([128, TSLICE], f32, tag="ga")
                        nc.scalar.activation(ga[:], pa[:], AF.Gelu,
                                             bias=b1c[:, ft:ft + 1])
                        nc.vector.tensor_copy(g1a[ft][:], ga[:])

            # ff = g1a @ W2, out = z + ff + b2
            with tc.tile_pool(name="ffw", bufs=3) as ffw, \
                    tc.tile_pool(name="ffps", bufs=1, space="PSUM") as ffps:
                pf0 = [ffps.tile([128, 512], f32, tag=f"pf0_{i}", name=f"pf0_{i}")
                       for i in range(ZT)]
                pf1 = [ffps.tile([128, 256], f32, tag=f"pf1_{i}", name=f"pf1_{i}")
                       for i in range(ZT)]
                for ft in range(FT):
                    w_ = ffw.tile([128, EMB], f32r, tag="w2t")
                    nc.sync.dma_start(w_[:], w2_d[128 * ft:128 * (ft + 1), :])
                    for i in range(ZT):
                        nc.tensor.matmul(
                            pf0[i][:],
                            g1a[ft][:, 128 * i:128 * (i + 1)],
                            w_[:, 0:512],
                            start=(ft == 0), stop=(ft == FT - 1))
                        nc.tensor.matmul(
                            pf1[i][:],
                            g1a[ft][:, 128 * i:128 * (i + 1)],
                            w_[:, 512:768],
                            start=(ft == 0), stop=(ft == FT - 1))
                for i in range(ZT):
                    o_ = p5s.tile([128, EMB], f32, tag="oadd")
                    nc.vector.tensor_add(o_[:, 0:512], pf0[i][:],
                                         z[i][:, 0:512])
                    nc.vector.tensor_add(o_[:, 512:768], pf1[i][:],
                                         z[i][:, 512:768])
                    nc.vector.tensor_add(o_[:], o_[:], b2_b[:])
                    nc.sync.dma_start(out_d[128 * i:128 * (i + 1), :], o_[:])


class _Runner:
    """Compiles the NEFF once and keeps a reusable jitted executable."""

    def __init__(self):
        import jax
        from jax.sharding import Mesh, PartitionSpec
        from jax.experimental.shard_map import shard_map
        from concourse import bass2jax
        from concourse import mybir

        bass2jax.install_neuronx_cc_hook()
        nc = _build_nc()
        self.nc = nc

        partition_name = (nc.partition_id_tensor.name
                          if nc.partition_id_tensor else None)
        in_names, out_names, out_avals, zero_outs = [], [], [], []
        for alloc in nc.m.functions[0].allocations:
            if not isinstance(alloc, mybir.MemoryLocationSet):
                continue
            name = alloc.memorylocations[0].name
            if alloc.kind == "ExternalInput":
                if name != partition_name:
                    in_names.append(name)
            elif alloc.kind == "ExternalOutput":
                shape = tuple(alloc.tensor_shape)
                dtype = mybir.dt.np(alloc.dtype)
                out_names.append(name)
                out_avals.append(jax.core.ShapedArray(shape, dtype))
                zero_outs.append(np.zeros(shape, dtype))
        self.in_names = list(in_names)
        self.out_names = out_names
        self.out_avals = out_avals
        self.zero_outs = zero_outs
        n_params = len(in_names)
        n_outs = len(out_avals)
        all_in_names = list(in_names) + list(out_names)
        if partition_name is not None:
            all_in_names.append(partition_name)

        from concourse.bass2jax import _bass_exec_p, partition_id_tensor

        def _body(*args):
            operands = list(args)
            if partition_name is not None:
                operands.append(partition_id_tensor())
            outs = _bass_exec_p.bind(
                *operands,
                out_avals=tuple(out_avals),
                in_names=tuple(all_in_names),
                out_names=tuple(out_names),
                lowering_input_output_aliases=(),
                sim_require_finite=True,
                sim_require_nnan=True,
                nc=nc,
            )
            return tuple(outs)

        devices = jax.devices()[:N_CORES]
        assert len(devices) == N_CORES
        mesh = Mesh(np.asarray(devices), ("core",))
        in_specs = (PartitionSpec("core"),) * (n_params + n_outs)
        out_specs = (PartitionSpec("core"),) * n_outs
        self.fn = jax.jit(
            shard_map(_body, mesh=mesh, in_specs=in_specs,
                      out_specs=out_specs, check_rep=False),
            donate_argnums=tuple(range(n_params, n_params + n_outs)),
            keep_unused=True)
        self.jax = jax

    def concat_inputs(self, in_maps):
        return [
            np.concatenate([np.asarray(in_maps[c][nm])
                            for c in range(N_CORES)], axis=0)
            for nm in self.in_names
        ]

    def zeros(self):
        return [np.zeros((N_CORES * z.shape[0], *z.shape[1:]), z.dtype)
                for z in self.zero_outs]

    def run_arrays(self, concat_in):
        return self.fn(*concat_in, *self.zeros())

    def run(self, in_maps):
        arrs = self.run_arrays(self.concat_inputs(in_maps))
        out = []
        for c in range(N_CORES):
            out.append({
                nm: np.asarray(arrs[i]).reshape(
                    N_CORES, *self.out_avals[i].shape)[c]
                for i, nm in enumerate(self.out_names)})
        return out


def _get_runner():
    global _RUNNER
    if _RUNNER is None:
        _RUNNER = _Runner()
    return _RUNNER


def _prep_inputs(x, Wq, Wk, Wv, Wo, bo, W1, b1, W2, b2, g1, beta1, g2, beta2):
    bf = ml_dtypes.bfloat16
    x = np.asarray(x, np.float32)
    in_maps = []
    w1f = np.ascontiguousarray(np.asarray(W1, np.float32))
    w2f = np.ascontiguousarray(np.asarray(W2, np.float32))
    def cols(v):
        v = np.asarray(v, np.float32)
        return np.ascontiguousarray(v.reshape(-1, 128).T)

    def row(v):
        return np.ascontiguousarray(np.asarray(v, np.float32).reshape(1, -1))

    vecs = dict(
        b1v=cols(b1), g1v=cols(g1), be1v=cols(beta1), g2v=cols(g2),
        be2v=cols(beta2), bov=row(bo), b2v=row(b2))
    wq_s = np.asarray(Wq, np.float32) * SCALE
    for c in range(N_CORES):
        b, g = divmod(c, 4)
        cs = slice(GW * g, GW * (g + 1))
        m = dict(
            xT=np.ascontiguousarray(x[b].T),
            xres=np.ascontiguousarray(x[b, TSLICE * g:TSLICE * (g + 1)]),
            wq=np.ascontiguousarray(wq_s[:, cs]).astype(bf),
            wk=np.ascontiguousarray(
                np.asarray(Wk, np.float32)[:, cs]).astype(bf),
            wv=np.ascontiguousarray(
                np.asarray(Wv, np.float32)[:, cs]).astype(bf),
            wo=np.ascontiguousarray(
                np.asarray(Wo, np.float32)[cs, :]).astype(bf),
            w1=w1f, w2=w2f, **vecs)
        in_maps.append(m)
    return in_maps


def kernel(**inputs):
    runner = _get_runner()
    in_maps = _prep_inputs(**inputs)
    res = runner.run(in_maps)
    out = np.empty((B, T, EMB), np.float32)
    for c in range(N_CORES):
        b, g = divmod(c, 4)
        out[b, TSLICE * g:TSLICE * (g + 1), :] = res[c]["out"]
    return out


# revision 12
# speedup vs baseline: 15.9783x; 15.9783x over previous
"""Trainium2 Bass kernel for a pre-norm transformer decoder layer.

Module: B=2, T=2048, EMB=768, 12 heads x head_dim 768 (QKV proj 768->9216),
causal attention, out_proj 9216->768, MLP 768->3072->768 (exact gelu),
pre-LN on both sublayers, residual adds.

Sharding over 8 NeuronCores: core c = (b, g) with b = c // 4 (batch),
g = c % 4 (head-group of 3 heads).  Each core:
  LN1 -> qT/kT (transposed proj) + v for its 3 heads -> causal attention
  -> partial out_proj (its heads' slice of Wo) -> on-device ReduceScatter
  (groups [0-3], [4-7]) so core (b,g) holds token rows [512g, 512g+512) of
  the attention sublayer output -> residual + LN2 + MLP on that slice
  -> returns [512, 768].  Host concatenates.

Numerics: attention pipeline in bf16 operands (fp32 psum accum), MLP
matmuls in float32r, layernorm / softmax / residuals in fp32.
"""

import numpy as np
import ml_dtypes

EMB = 768
HEADS = 12
HD = 768
QKV = 9216
FF = 3072
T = 2048
B = 2
N_CORES = 8
HPG = 3              # heads per group
GW = HPG * HD        # 2304, group slice of QKV dim
EPS = 1e-5
SCALE = 1.0 / 96.0   # 1/sqrt(9216)
NEG = -30000.0       # additive causal mask value (exp -> 0)
TSLICE = T // 4      # 512 tokens per core after reduce-scatter

ET = EMB // 128      # 6   e-tiles
CT = GW // 128       # 18  c-tiles per group
TT = T // 128        # 16  t-tiles
TC = T // 512        # 4   t-chunks
FT = FF // 128       # 24  f-tiles
ZT = TSLICE // 128   # 4   z-tiles

_RUNNER = None


def _mask_ext_np():
    """[128, 896] extended causal mask.  Pattern k (k=0..3) for an S^T tile
    [s=128k+i, t=j (512 wide)] is mask_ext[:, 384-128k : 896-128k]:
    allowed (0.0) iff 128k + i <= j, else NEG."""
    m = np.full((128, 896), NEG, np.float32)
    for i in range(128):
        m[i, 384 + i:] = 0.0
    return m


def _build_nc():
    import concourse.bacc as bacc
    import concourse.tile as tile
    from concourse import mybir

    f32 = mybir.dt.float32
    bf16 = mybir.dt.bfloat16
    f32r = mybir.dt.float32r
    AF = mybir.ActivationFunctionType
    OP = mybir.AluOpType

    nc = bacc.Bacc("TRN2", target_bir_lowering=False, debug=False,
                   num_devices=N_CORES)

    # ---- I/O ----
    xT_d = nc.dram_tensor("xT", [EMB, T], f32, kind="ExternalInput")
    xres_d = nc.dram_tensor("xres", [TSLICE, EMB], f32, kind="ExternalInput")
    wq_d = nc.dram_tensor("wq", [EMB, GW], bf16, kind="ExternalInput")
    wk_d = nc.dram_tensor("wk", [EMB, GW], bf16, kind="ExternalInput")
    wv_d = nc.dram_tensor("wv", [EMB, GW], bf16, kind="ExternalInput")
    wo_d = nc.dram_tensor("wo", [GW, EMB], bf16, kind="ExternalInput")
    w1_d = nc.dram_tensor("w1", [EMB, FF], f32r, kind="ExternalInput")
    w2_d = nc.dram_tensor("w2", [FF, EMB], f32r, kind="ExternalInput")
    # vectors pre-reshaped on host: [128, n] with col j = v[128j:128(j+1)]
    b1_d = nc.dram_tensor("b1v", [128, FT], f32, kind="ExternalInput")
    g1_d = nc.dram_tensor("g1v", [128, ET], f32, kind="ExternalInput")
    be1_d = nc.dram_tensor("be1v", [128, ET], f32, kind="ExternalInput")
    g2_d = nc.dram_tensor("g2v", [128, ET], f32, kind="ExternalInput")
    be2_d = nc.dram_tensor("be2v", [128, ET], f32, kind="ExternalInput")
    bo_d = nc.dram_tensor("bov", [1, EMB], f32, kind="ExternalInput")
    b2_d = nc.dram_tensor("b2v", [1, EMB], f32, kind="ExternalInput")
    out_d = nc.dram_tensor("out", [TSLICE, EMB], f32, kind="ExternalOutput")

    # ---- consts / internal DRAM ----
    cmask_c = nc.inline_tensor(_mask_ext_np(), name="cmask")
    ident_c = nc.inline_tensor(np.eye(128, dtype=np.float32), name="ident")
    ones_f_c = nc.inline_tensor(np.ones((128, 1), np.float32), name="ones_f")
    ones_b_c = nc.inline_tensor(
        np.ones((128, 1), dtype=ml_dtypes.bfloat16), name="ones_b")

    vsc_d = nc.dram_tensor("vsc", [HPG, T, HD], bf16)        # v scratch
    ysc_d = nc.dram_tensor("ysc", [CT, 128, T], bf16)        # yT scratch
    rsin_d = nc.dram_tensor("rs_in", [T, EMB], f32)          # out_proj partial
    rsout_d = nc.dram_tensor("rs_out", [TSLICE, EMB], f32)   # reduced slice

    with tile.TileContext(nc) as tc:
        _build_body(nc, tc, mybir, locals())
    nc.compile()
    return nc


def _build_body(nc, tc, mybir, d):
    import contextlib

    f32 = mybir.dt.float32
    bf16 = mybir.dt.bfloat16
    f32r = mybir.dt.float32r
    AF = mybir.ActivationFunctionType
    OP = mybir.AluOpType

    xT_d = d["xT_d"]; xres_d = d["xres_d"]
    wq_d = d["wq_d"]; wk_d = d["wk_d"]; wv_d = d["wv_d"]; wo_d = d["wo_d"]
    w1_d = d["w1_d"]; w2_d = d["w2_d"]
    b1_d = d["b1_d"]; g1_d = d["g1_d"]; be1_d = d["be1_d"]
    g2_d = d["g2_d"]; be2_d = d["be2_d"]; bo_d = d["bo_d"]; b2_d = d["b2_d"]
    out_d = d["out_d"]
    cmask_c = d["cmask_c"]; ident_c = d["ident_c"]
    ones_f_c = d["ones_f_c"]; ones_b_c = d["ones_b_c"]
    vsc_d = d["vsc_d"]; ysc_d = d["ysc_d"]
    rsin_d = d["rsin_d"]; rsout_d = d["rsout_d"]

    es = contextlib.ExitStack()
    with es:
        const = es.enter_context(tc.tile_pool(name="const", bufs=1))
        rows = es.enter_context(tc.tile_pool(name="rows", bufs=1))

        # ---------- constants ----------
        cmask = const.tile([128, 896], f32, tag="cmask")
        nc.sync.dma_start(cmask[:], cmask_c[:, :])
        ident = const.tile([128, 128], f32, tag="ident")
        nc.sync.dma_start(ident[:], ident_c[:, :])
        ones_b = const.tile([128, 1], bf16, tag="ones_b")
        nc.sync.dma_start(ones_b[:], ones_b_c[:, :])

        def load_cols(dram, n, tag):
            # host pre-reshaped [128, n], col j = v[128j:128j+128]
            t_ = const.tile([128, n], f32, tag=tag, name=tag)
            nc.sync.dma_start(t_[:, :], dram[:, :])
            return t_

        g1c = load_cols(g1_d, ET, "g1c")
        be1c = load_cols(be1_d, ET, "be1c")
        g2c = load_cols(g2_d, ET, "g2c")
        be2c = load_cols(be2_d, ET, "be2c")
        b1c = load_cols(b1_d, FT, "b1c")

        # ---------- P0: LN1 over full batch sequence ----------
        ht_pool = es.enter_context(tc.tile_pool(name="ht", bufs=1))
        ht = [ht_pool.tile([128, T], bf16, tag=f"ht{j}", name=f"ht{j}")
          for j in range(ET)]

        with tc.tile_pool(name="p0", bufs=1) as p0, \
                tc.tile_pool(name="p0ps", bufs=1, space="PSUM") as p0ps:
            xt = []
            for j in range(ET):
                t_ = p0.tile([128, T], f32, tag=f"xt{j}")
                nc.sync.dma_start(t_[:], xT_d[128 * j:128 * (j + 1), :])
                xt.append(t_)

            mu_ps = [p0ps.tile([1, 512], f32, tag=f"mups{c}", name=f"mups{c}")
                     for c in range(TC)]
            ss_ps = [p0ps.tile([1, 512], f32, tag=f"ssps{c}", name=f"ssps{c}")
                     for c in range(TC)]
            for j in range(ET):
                xb = p0.tile([128, T], bf16, tag="xb")
                nc.vector.tensor_copy(xb[:], xt[j][:])
                sq = p0.tile([128, T], bf16, tag="sq")
                nc.scalar.square(sq[:], xt[j][:])
                for c in range(TC):
                    cs = slice(512 * c, 512 * (c + 1))
                    nc.tensor.matmul(
                        mu_ps[c][:], ones_b[:], xb[:, cs],
                        start=(j == 0), stop=(j == ET - 1))
                    nc.tensor.matmul(
                        ss_ps[c][:], ones_b[:], sq[:, cs],
                        start=(j == 0), stop=(j == ET - 1))

            mu_b = p0.tile([128, T], f32, tag="mu_b")
            rstd_b = p0.tile([128, T], f32, tag="rstd_b")
            for c in range(TC):
                cs = slice(512 * c, 512 * (c + 1))
                mrow = rows.tile([1, 512], f32, tag="r_mu")
                nc.scalar.mul(mrow[:], mu_ps[c][:], 1.0 / EMB)
                vrow = rows.tile([1, 512], f32, tag="r_var")
                nc.scalar.mul(vrow[:], ss_ps[c][:], 1.0 / EMB)
                nc.gpsimd.partition_broadcast(mu_b[:, cs], mrow[:])
                nc.vector.tensor_mul(mrow[:], mrow[:], mrow[:])
                nc.vector.tensor_sub(vrow[:], vrow[:], mrow[:])
                nc.vector.tensor_scalar_add(vrow[:], vrow[:], EPS)
                nc.scalar.activation(vrow[:], vrow[:], AF.Sqrt)
                nc.vector.reciprocal(vrow[:], vrow[:])
                nc.gpsimd.partition_broadcast(rstd_b[:, cs], vrow[:])

            for j in range(ET):
                t1 = p0.tile([128, T], f32, tag="ln1t")
                nc.vector.tensor_sub(t1[:], xt[j][:], mu_b[:])
                nc.vector.tensor_mul(t1[:], t1[:], rstd_b[:])
                nc.vector.tensor_scalar(
                    out=ht[j][:], in0=t1[:],
                    scalar1=g1c[:, j:j + 1], scalar2=be1c[:, j:j + 1],
                    op0=OP.mult, op1=OP.add)

        # ---------- attention region ----------
        with tc.tile_pool(name="att", bufs=1) as att, \
                tc.tile_pool(name="attw", bufs=2) as attw, \
                tc.tile_pool(name="attps", bufs=1, space="PSUM") as attps:

            def acc512(): return attps.tile([128, 512], f32,
                                             tag="acc512", bufs=2,
                                             name="acc512")
            def acc256(): return attps.tile([128, 256], f32,
                                             tag="acc512", bufs=2,
                                             name="acc256")

            qT = [att.tile([128, T], bf16, tag=f"qT{i}", name=f"qT{i}")
                  for i in range(ET)]
            kT = [att.tile([128, T], bf16, tag=f"kT{i}", name=f"kT{i}")
                  for i in range(ET)]

            for h in range(HPG):
                hs = slice(HD * h, HD * (h + 1))

                # --- projections ---
                for (dst, wsrc) in ((qT, wq_d), (kT, wk_d)):
                    wt = []
                    for j in range(ET):
                        w_ = attw.tile([128, HD], bf16, tag=f"wph{j}")
                        nc.sync.dma_start(
                            w_[:], wsrc[128 * j:128 * (j + 1), hs])
                        wt.append(w_)
                    for ci in range(ET):
                        csl = slice(128 * ci, 128 * (ci + 1))
                        for tp in range(2):
                            pa = acc512()
                            pb = acc512()
                            for j in range(ET):
                                nc.tensor.matmul(
                                    pa[:], wt[j][:, csl],
                                    ht[j][:, 1024 * tp:1024 * tp + 512],
                                    start=(j == 0), stop=(j == ET - 1))
                                nc.tensor.matmul(
                                    pb[:], wt[j][:, csl],
                                    ht[j][:, 1024 * tp + 512:1024 * (tp + 1)],
                                    start=(j == 0), stop=(j == ET - 1))
                            nc.vector.tensor_copy(
                                dst[ci][:, 1024 * tp:1024 * tp + 512], pa[:])
                            nc.vector.tensor_copy(
                                dst[ci][:, 1024 * tp + 512:1024 * (tp + 1)],
                                pb[:])

                wvh = []
                for j in range(ET):
                    w_ = attw.tile([128, HD], bf16, tag=f"wph{j}")
                    nc.sync.dma_start(w_[:], wv_d[128 * j:128 * (j + 1), hs])
                    wvh.append(w_)
                for i in range(TT):
                    ts_ = slice(128 * i, 128 * (i + 1))
                    p0_ = acc512()
                    p1_ = acc256()
                    for j in range(ET):
                        nc.tensor.matmul(p0_[:], ht[j][:, ts_],
                                         wvh[j][:, 0:512],
                                         start=(j == 0), stop=(j == ET - 1))
                        nc.tensor.matmul(p1_[:], ht[j][:, ts_],
                                         wvh[j][:, 512:768],
                                         start=(j == 0), stop=(j == ET - 1))
                    vt_ = attw.tile([128, HD], bf16, tag="vstage")
                    nc.vector.tensor_copy(vt_[:, 0:512], p0_[:])
                    nc.vector.tensor_copy(vt_[:, 512:768], p1_[:])
                    nc.sync.dma_start(vsc_d[h, ts_, :], vt_[:])

                # --- attention ---
                for tcn in range(TC):
                    ns = 4 * (tcn + 1)
                    tsl = slice(512 * tcn, 512 * (tcn + 1))
                    pts = []
                    for si in range(ns):
                        ssl = slice(128 * si, 128 * (si + 1))
                        ps_ = acc512()
                        for di in range(ET):
                            nc.tensor.matmul(
                                ps_[:], kT[di][:, ssl], qT[di][:, tsl],
                                start=(di == 0), stop=(di == ET - 1))
                        k_ = si - 4 * tcn
                        if k_ >= 0:
                            mo = 384 - 128 * k_
                            nc.vector.tensor_add(
                                ps_[:], ps_[:], cmask[:, mo:mo + 512])
                        pt_ = att.tile([128, 512], bf16, tag=f"pt{si}")
                        nc.scalar.activation(pt_[:], ps_[:], AF.Exp)
                        pts.append(pt_)
                    pden = attps.tile([1, 512], f32, tag="accY", bufs=6)
                    for si in range(ns):
                        nc.tensor.matmul(pden[:], ones_b[:], pts[si][:],
                                         start=(si == 0), stop=(si == ns - 1))
                    den = rows.tile([1, 512], f32, tag="r_den")
                    nc.vector.reciprocal(den[:], pden[:])
                    rb = attw.tile([128, 512], f32, tag="rb")
                    nc.gpsimd.partition_broadcast(rb[:], den[:])

                    pys = [attps.tile([128, 512], f32, tag="accY", bufs=6,
                                      name=f"py{_d}") for _d in range(ET)]
                    for si in range(ns):
                        ssl = slice(128 * si, 128 * (si + 1))
                        vt_ = attw.tile([128, HD], bf16, tag="vt", bufs=3)
                        nc.sync.dma_start(vt_[:], vsc_d[h, ssl, :])
                        for di in range(ET):
                            nc.tensor.matmul(
                                pys[di][:], vt_[:, 128 * di:128 * (di + 1)],
                                pts[si][:],
                                start=(si == 0), stop=(si == ns - 1))
                    for di in range(ET):
                        yst = attw.tile([128, 512], bf16, tag="ystg")
                        nc.vector.tensor_tensor(
                            out=yst[:], in0=pys[di][:], in1=rb[:], op=OP.mult)
                        nc.sync.dma_start(
                            ysc_d[ET * h + di, :, tsl], yst[:])

        # ---------- out_proj partial -> rs_in ----------
        with tc.tile_pool(name="p3", bufs=1) as p3, \
                tc.tile_pool(name="p3s", bufs=2) as p3s, \
                tc.tile_pool(name="p3ps", bufs=2, space="PSUM") as p3ps:
            wot = []
            for ci in range(CT):
                w_ = p3.tile([128, EMB], bf16, tag=f"wot{ci}")
                nc.sync.dma_start(w_[:], wo_d[128 * ci:128 * (ci + 1), :])
                wot.append(w_)
            for i in range(TT):
                ts_ = slice(128 * i, 128 * (i + 1))
                p0_ = p3ps.tile([128, 512], f32, tag="po0")
                p1_ = p3ps.tile([128, 256], f32, tag="po1")
                for ci in range(CT):
                    y_ = p3s.tile([128, 128], bf16, tag=f"yld{ci % 3}")
                    nc.sync.dma_start(y_[:], ysc_d[ci, :, ts_])
                    nc.tensor.matmul(p0_[:], y_[:], wot[ci][:, 0:512],
                                     start=(ci == 0), stop=(ci == CT - 1))
                    nc.tensor.matmul(p1_[:], y_[:], wot[ci][:, 512:768],
                                     start=(ci == 0), stop=(ci == CT - 1))
                ost = p3s.tile([128, EMB], f32, tag="ost")
                nc.vector.tensor_copy(ost[:, 0:512], p0_[:])
                nc.vector.tensor_copy(ost[:, 512:768], p1_[:])
                nc.sync.dma_start(rsin_d[ts_, :], ost[:])

        # ---------- reduce-scatter ----------
        nc.gpsimd.collective_compute(
            "ReduceScatter", OP.add,
            replica_groups=[[0, 1, 2, 3], [4, 5, 6, 7]],
            ins=[rsin_d.ap().opt()], outs=[rsout_d.ap().opt()])

        # ---------- residual + LN2 + MLP ----------
        with tc.tile_pool(name="p5", bufs=1) as p5, \
                tc.tile_pool(name="p5s", bufs=2) as p5s:
            bo_row = rows.tile([1, EMB], f32, tag="bo_row")
            nc.sync.dma_start(bo_row[:, :], bo_d[:, :])
            b2_row = rows.tile([1, EMB], f32, tag="b2_row")
            nc.sync.dma_start(b2_row[:, :], b2_d[:, :])
            bo_b = p5.tile([128, EMB], f32, tag="bo_b")
            nc.gpsimd.partition_broadcast(bo_b[:], bo_row[:])
            b2_b = p5.tile([128, EMB], f32, tag="b2_b")
            nc.gpsimd.partition_broadcast(b2_b[:], b2_row[:])

            z = []
            for i in range(ZT):
                ts_ = slice(128 * i, 128 * (i + 1))
                rl = p5s.tile([128, EMB], f32, tag="rsld")
                nc.sync.dma_start(rl[:], rsout_d[ts_, :])
                xr = p5s.tile([128, EMB], f32, tag="xrld")
                nc.sync.dma_start(xr[:], xres_d[ts_, :])
                z_ = p5.tile([128, EMB], f32, tag=f"z{i}")
                nc.vector.tensor_add(z_[:], rl[:], xr[:])
                nc.vector.tensor_add(z_[:], z_[:], bo_b[:])
                z.append(z_)

            h2 = [p5.tile([128, TSLICE], f32r, tag=f"h2{j}", name=f"h2{j}")
                  for j in range(ET)]
            with tc.tile_pool(name="ln2", bufs=1) as ln2, \
                    tc.tile_pool(name="ln2ps", bufs=1, space="PSUM") as ln2ps:
                zTt = [ln2.tile([128, TSLICE], f32, tag=f"zT{j}", name=f"zT{j}")
                       for j in range(ET)]
                for i in range(ZT):
                    for j in range(ET):
                        pt_ = ln2ps.tile([128, 128], f32, tag="ptr", bufs=2)
                        nc.tensor.transpose(
                            pt_[:], z[i][:, 128 * j:128 * (j + 1)], ident[:])
                        nc.vector.tensor_copy(
                            zTt[j][:, 128 * i:128 * (i + 1)], pt_[:])
                pmu2 = ln2ps.tile([1, 512], f32, tag="pmu2")
                pss2 = ln2ps.tile([1, 512], f32, tag="pss2")
                for j in range(ET):
                    zb = ln2.tile([128, TSLICE], bf16, tag="zb", bufs=2)
                    nc.vector.tensor_copy(zb[:], zTt[j][:])
                    sq = ln2.tile([128, TSLICE], bf16, tag="sq2", bufs=2)
                    nc.scalar.square(sq[:], zTt[j][:])
                    nc.tensor.matmul(pmu2[:], ones_b[:], zb[:],
                                     start=(j == 0), stop=(j == ET - 1))
                    nc.tensor.matmul(pss2[:], ones_b[:], sq[:],
                                     start=(j == 0), stop=(j == ET - 1))
                mrow = rows.tile([1, 512], f32, tag="r_mu")
                nc.scalar.mul(mrow[:], pmu2[:], 1.0 / EMB)
                vrow = rows.tile([1, 512], f32, tag="r_var")
                nc.scalar.mul(vrow[:], pss2[:], 1.0 / EMB)
                mu2_b = ln2.tile([128, TSLICE], f32, tag="mu2_b")
                nc.gpsimd.partition_broadcast(mu2_b[:], mrow[:])
                nc.vector.tensor_mul(mrow[:], mrow[:], mrow[:])
                nc.vector.tensor_sub(vrow[:], vrow[:], mrow[:])
                nc.vector.tensor_scalar_add(vrow[:], vrow[:], EPS)
                nc.scalar.activation(vrow[:], vrow[:], AF.Sqrt)
                nc.vector.reciprocal(vrow[:], vrow[:])
                rstd2_b = ln2.tile([128, TSLICE], f32, tag="rstd2_b")
                nc.gpsimd.partition_broadcast(rstd2_b[:], vrow[:])
                for j in range(ET):
                    t1 = ln2.tile([128, TSLICE], f32, tag="ln2t", bufs=2)
                    nc.vector.tensor_sub(t1[:], zTt[j][:], mu2_b[:])
                    nc.vector.tensor_mul(t1[:], t1[:], rstd2_b[:])
                    nc.vector.tensor_scalar(
                        out=h2[j][:], in0=t1[:],
                        scalar1=g2c[:, j:j + 1], scalar2=be2c[:, j:j + 1],
                        op0=OP.mult, op1=OP.add)

            # a1 = h2 @ W1 + b1 -> gelu -> g1a
            g1a = [p5.tile([128, TSLICE], f32r, tag=f"g1a{ft}",
                           name=f"g1a{ft}") for ft in range(FT)]
            with tc.tile_pool(name="a1w", bufs=2) as a1w, \
                    tc.tile_pool(name="a1ps", bufs=2, space="PSUM") as a1ps:
                for fc in range(FF // 512):
                    w1t = []
                    for j in range(ET):
                        w_ = a1w.tile([128, 512], f32r, tag=f"w1t{j}")
                        nc.sync.dma_start(
                            w_[:], w1_d[128 * j:128 * (j + 1),
                                        512 * fc:512 * (fc + 1)])
                        w1t.append(w_)
                    for sub in range(4):
                        ft = 4 * fc + sub
                        pa = a1ps.tile([128, 512], f32, tag="pa1")
                        for j in range(ET):
                            nc.tensor.matmul(
                                pa[:],
                                w1t[j][:, 128 * sub:128 * (sub + 1)],
                                h2[j][:],
                                start=(j == 0), stop=(j == ET - 1))
                        ga = a1w.tile([128, TSLICE], f32, tag="ga")
                        nc.scalar.activation(ga[:], pa[:], AF.Gelu,
                                             bias=b1c[:, ft:ft + 1])
                        nc.vector.tensor_copy(g1a[ft][:], ga[:])

            # ff = g1a @ W2, out = z + ff + b2
            with tc.tile_pool(name="ffw", bufs=3) as ffw, \
                    tc.tile_pool(name="ffps", bufs=1, space="PSUM") as ffps:
                pf0 = [ffps.tile([128, 512], f32, tag=f"pf0_{i}", name=f"pf0_{i}")
                       for i in range(ZT)]
                pf1 = [ffps.tile([128, 256], f32, tag=f"pf1_{i}", name=f"pf1_{i}")
                       for i in range(ZT)]
                for ft in range(FT):
                    w_ = ffw.tile([128, EMB], f32r, tag="w2t")
                    nc.sync.dma_start(w_[:], w2_d[128 * ft:128 * (ft + 1), :])
                    for i in range(ZT):
                        nc.tensor.matmul(
                            pf0[i][:],
                            g1a[ft][:, 128 * i:128 * (i + 1)],
                            w_[:, 0:512],
                            start=(ft == 0), stop=(ft == FT - 1))
                        nc.tensor.matmul(
                            pf1[i][:],
                            g1a[ft][:, 128 * i:128 * (i + 1)],
                            w_[:, 512:768],
                            start=(ft == 0), stop=(ft == FT - 1))
                for i in range(ZT):
                    o_ = p5s.tile([128, EMB], f32, tag="oadd")
                    nc.vector.tensor_add(o_[:, 0:512], pf0[i][:],
                                         z[i][:, 0:512])
                    nc.vector.tensor_add(o_[:, 512:768], pf1[i][:],
                                         z[i][:, 512:768])
                    nc.vector.tensor_add(o_[:], o_[:], b2_b[:])
                    nc.sync.dma_start(out_d[128 * i:128 * (i + 1), :], o_[:])


class _Runner:
    """Compiles the NEFF once and keeps a reusable jitted executable."""

    def __init__(self):
        import jax
        from jax.sharding import Mesh, PartitionSpec
        from jax.experimental.shard_map import shard_map
        from concourse import bass2jax
        from concourse import mybir

        bass2jax.install_neuronx_cc_hook()
        nc = _build_nc()
        self.nc = nc

        partition_name = (nc.partition_id_tensor.name
                          if nc.partition_id_tensor else None)
        in_names, out_names, out_avals, zero_outs = [], [], [], []
        for alloc in nc.m.functions[0].allocations:
            if not isinstance(alloc, mybir.MemoryLocationSet):
                continue
            name = alloc.memorylocations[0].name
            if alloc.kind == "ExternalInput":
                if name != partition_name:
                    in_names.append(name)
            elif alloc.kind == "ExternalOutput":
                shape = tuple(alloc.tensor_shape)
                dtype = mybir.dt.np(alloc.dtype)
                out_names.append(name)
                out_avals.append(jax.core.ShapedArray(shape, dtype))
                zero_outs.append(np.zeros(shape, dtype))
        self.in_names = list(in_names)
        self.out_names = out_names
        self.out_avals = out_avals
        self.zero_outs = zero_outs
        n_params = len(in_names)
        n_outs = len(out_avals)
        all_in_names = list(in_names) + list(out_names)
        if partition_name is not None:
            all_in_names.append(partition_name)

        from concourse.bass2jax import _bass_exec_p, partition_id_tensor

        def _body(*args):
            operands = list(args)
            if partition_name is not None:
                operands.append(partition_id_tensor())
            outs = _bass_exec_p.bind(
                *operands,
                out_avals=tuple(out_avals),
                in_names=tuple(all_in_names),
                out_names=tuple(out_names),
                lowering_input_output_aliases=(),
                sim_require_finite=True,
                sim_require_nnan=True,
                nc=nc,
            )
            return tuple(outs)

        devices = jax.devices()[:N_CORES]
        assert len(devices) == N_CORES
        mesh = Mesh(np.asarray(devices), ("core",))
        self.mesh = mesh
        in_specs = (PartitionSpec("core"),) * (n_params + n_outs)
        out_specs = (PartitionSpec("core"),) * n_outs
        self.fn = jax.jit(
            shard_map(_body, mesh=mesh, in_specs=in_specs,
                      out_specs=out_specs, check_rep=False),
            donate_argnums=tuple(range(n_params, n_params + n_outs)),
            keep_unused=True)
        self.jax = jax

    def concat_inputs(self, in_maps):
        return [
            np.concatenate([np.asarray(in_maps[c][nm])
                            for c in range(N_CORES)], axis=0)
            for nm in self.in_names
        ]

    def zeros(self):
        return [np.zeros((N_CORES * z.shape[0], *z.shape[1:]), z.dtype)
                for z in self.zero_outs]

    def run_arrays(self, concat_in):
        return self.fn(*concat_in, *self.zeros())

    def run(self, in_maps):
        arrs = self.run_arrays(self.concat_inputs(in_maps))
        out = []
        for c in range(N_CORES):
            out.append({
                nm: np.asarray(arrs[i]).reshape(
                    N_CORES, *self.out_avals[i].shape)[c]
                for i, nm in enumerate(self.out_names)})
        return out


def _get_runner():
    global _RUNNER
    if _RUNNER is None:
        _RUNNER = _Runner()
    return _RUNNER


def _prep_inputs(x, Wq, Wk, Wv, Wo, bo, W1, b1, W2, b2, g1, beta1, g2, beta2):
    bf = ml_dtypes.bfloat16
    x = np.asarray(x, np.float32)
    in_maps = []
    w1f = np.ascontiguousarray(np.asarray(W1, np.float32))
    w2f = np.ascontiguousarray(np.asarray(W2, np.float32))
    def cols(v):
        v = np.asarray(v, np.float32)
        return np.ascontiguousarray(v.reshape(-1, 128).T)

    def row(v):
        return np.ascontiguousarray(np.asarray(v, np.float32).reshape(1, -1))

    vecs = dict(
        b1v=cols(b1), g1v=cols(g1), be1v=cols(beta1), g2v=cols(g2),
        be2v=cols(beta2), bov=row(bo), b2v=row(b2))
    wq_s = np.asarray(Wq, np.float32) * SCALE
    for c in range(N_CORES):
        b, g = divmod(c, 4)
        cs = slice(GW * g, GW * (g + 1))
        m = dict(
            xT=np.ascontiguousarray(x[b].T),
            xres=np.ascontiguousarray(x[b, TSLICE * g:TSLICE * (g + 1)]),
            wq=np.ascontiguousarray(wq_s[:, cs]).astype(bf),
            wk=np.ascontiguousarray(
                np.asarray(Wk, np.float32)[:, cs]).astype(bf),
            wv=np.ascontiguousarray(
                np.asarray(Wv, np.float32)[:, cs]).astype(bf),
            wo=np.ascontiguousarray(
                np.asarray(Wo, np.float32)[cs, :]).astype(bf),
            w1=w1f, w2=w2f, **vecs)
        in_maps.append(m)
    return in_maps


def kernel(**inputs):
    runner = _get_runner()
    in_maps = _prep_inputs(**inputs)
    res = runner.run(in_maps)
    out = np.empty((B, T, EMB), np.float32)
    for c in range(N_CORES):
        b, g = divmod(c, 4)
        out[b, TSLICE * g:TSLICE * (g + 1), :] = res[c]["out"]
    return out
